# revision 1
# baseline (speedup 1.0000x reference)
"""Multi-head self-attention (B=2, N=2048, D=1024, H=16, dh=64) on 8 trn2 cores.

Sharding: core c -> batch b = c // 4, head-group hg = c % 4 (4 heads per core).
Each core computes partial = Attn_{heads hg}(x_b) @ Wo[rows hg] (+ bo on hg==0);
the host sums the 4 partials per batch (the unshard step).

Per-core pipeline (all matmuls in float32r = full-rate fp32 on the PE):
  1. PE-transpose x_b -> xT (D on partitions)
  2. qT/kT projections (heads on partitions), v natural; v augmented with a
     ones column per head so the attn@v matmul also emits softmax denominators
  3. per head pair: scoresT = kT-blocks.T @ qT-blocks with the two heads
     packed in disjoint PE row groups (concurrent on silicon), exp on ScalarE
     (softmax scale folded into the activation's free affine), ctx
     accumulation over key tiles in PSUM
  4. one DVE reciprocal over all 16 denominator rows, DRAM-bounce broadcast,
     normalize ctxT in place
  5. out = ctxT.T @ Wo_c + bo via PSUM accumulation (bias added with a K=1
     ones matmul), staged through SBUF in 4-row-tile batches
"""

import numpy as np

B, N, D = 2, 2048, 1024
H, DH = 16, 64
HPC = 4                # heads per core
CS = HPC * DH          # 256 = per-core slice of the inner dim
NCORES = 8
SCALE = DH ** -0.5

NT = N // 128          # 16 token tiles
KT = D // 128          # 8 contraction tiles
NIB = N // 512         # 4 query blocks
VW = DH + 1            # 65: v plus ones column

_CACHE = {}


def _build_nc(reps=1):
    import concourse.bass as bass
    import concourse.bacc as bacc
    import concourse.mybir as mybir
    import concourse.tile as tile
    from contextlib import ExitStack

    f32 = mybir.dt.float32
    f32r = mybir.dt.float32r
    PSUM = bass.MemorySpace.PSUM
    Exp = mybir.ActivationFunctionType.Exp

    nc = bacc.Bacc()

    x_d = nc.dram_tensor("x", [N, D], f32, kind="ExternalInput")
    wq_d = nc.dram_tensor("wq", [D, CS], f32r, kind="ExternalInput")
    wk_d = nc.dram_tensor("wk", [D, CS], f32r, kind="ExternalInput")
    wv_d = nc.dram_tensor("wv", [D, CS], f32r, kind="ExternalInput")
    wo_d = nc.dram_tensor("wo", [CS, D], f32r, kind="ExternalInput")
    bo_d = nc.dram_tensor("bo", [1, D], f32r, kind="ExternalInput")
    out_d = nc.dram_tensor("out", [N, D], f32, kind="ExternalOutput")
    den_d = nc.dram_tensor("den_scratch", [16, 512], f32r)
    den_d2 = nc.dram_tensor("den_scratch2", [16, 512], f32r)

    ident_d = nc.inline_tensor(np.eye(128, dtype=np.float32), name="ident")

    # grouped views for batched DMAs
    x_g = x_d.rearrange("(g j p) d -> g p j d", p=128, j=4)        # [4][128,4,1024]
    wq_g = wq_d.rearrange("(k p) c -> p k c", p=128)               # [128,8,256]
    wk_g = wk_d.rearrange("(k p) c -> p k c", p=128)
    wv_g = wv_d.rearrange("(k p) c -> p k c", p=128)
    wo_g = wo_d.rearrange("(k p) c -> p k c", p=128)               # [128,2,1024]
    out_g2 = out_d.rearrange("(q i p) e -> q p i e", p=128, i=2)   # [8][128,2,1024]
    den_r = den_d.rearrange("(g r) c -> g (r c)", g=4)             # [4,2048]

    with tile.TileContext(nc) as tc:
      for _rep in range(reps):
       with ExitStack() as es:
             singles = es.enter_context(tc.tile_pool(name="singles", bufs=1))

             ones_f32 = singles.tile([1, 128], f32, tag="ones32")
             nc.vector.memset(ones_f32, 1.0)
             ones_col = singles.tile([1, 128], f32r, tag="ones")
             nc.vector.tensor_copy(ones_col, ones_f32)
             ones4 = singles.tile([128, HPC, 1], f32, tag="ones4")
             nc.vector.memset(ones4, 1.0)

             wq_sb = singles.tile([128, KT, CS], f32r, tag="wq")
             wk_sb = singles.tile([128, KT, CS], f32r, tag="wk")
             wv_sb = singles.tile([128, KT, CS], f32r, tag="wv")
             wop = es.enter_context(tc.tile_pool(name="wop", bufs=1))
             wo_sb = wop.tile([128, 2, D], f32r, tag="wo")
             bo_sb = wop.tile([1, D], f32r, tag="bo")

             qT = [singles.tile([128, N], f32r, tag=f"qT{p}", name=f"qT{p}") for p in range(2)]
             kTt = [singles.tile([128, N], f32r, tag=f"kT{p}", name=f"kT{p}") for p in range(2)]
             vA = [singles.tile([128, HPC * VW], f32r, tag=f"v{t}", name=f"v{t}") for t in range(NT)]
             ctxT = [singles.tile([128, N], f32r, tag=f"ctxT{p}", name=f"ctxT{p}") for p in range(2)]
             den_all = [singles.tile([8, 512], f32r, tag=f"den{p}", name=f"den{p}") for p in range(2)]
             den_rec = [singles.tile([8, 512], f32r, tag=f"denr{p}", name=f"denr{p}") for p in range(2)]

             # ---- phase 1: transpose x; qk-p0; v[0:4] ----
             # xT and the projection PSUM pool survive into pair-0 attention so
             # the pair-1 q/k projections and remaining v tiles can run in PE
             # gaps while ScalarE chews pair-0 exponentials.
             xps = ExitStack()
             xT_pool = xps.enter_context(tc.tile_pool(name="xTp", bufs=KT))
             pj_ps = xps.enter_context(tc.tile_pool(name="pjps", bufs=2, space=PSUM))
             xT = [xT_pool.tile([128, N], f32r, tag="xT", name="xT") for _ in range(KT)]

             with ExitStack() as pes:
                 idp = pes.enter_context(tc.tile_pool(name="idp", bufs=1))
                 x_pool = pes.enter_context(tc.tile_pool(name="xp", bufs=2))
                 tp_ps = pes.enter_context(tc.tile_pool(name="tpps", bufs=2, space=PSUM))

                 ident = idp.tile([128, 128], f32, tag="ident")
                 nc.sync.dma_start(out=ident, in_=ident_d[:, :])

                 wdma = [
                     lambda: nc.sync.dma_start(out=wq_sb, in_=wq_g),
                     lambda: nc.sync.dma_start(out=wk_sb, in_=wk_g),
                     lambda: nc.sync.dma_start(out=wv_sb, in_=wv_g),
                     lambda: nc.sync.dma_start(out=wo_sb, in_=wo_g),
                     lambda: nc.sync.dma_start(out=bo_sb, in_=bo_d[:, :]),
                 ]
                 for g in range(NT // 4):  # 4 groups of 4 token tiles
                     xt = x_pool.tile([128, 4, D], f32, tag="x", name="xt")
                     nc.sync.dma_start(out=xt, in_=x_g[g])
                     if g > 0:
                         for w in wdma[(g - 1) * 2:g * 2]:
                             w()
                         if g == 3:
                             wdma[4]()
                     for dh in range(KT // 2):
                         ps = tp_ps.tile([128, 2, 512], f32, tag="tp", name="tp")
                         for dj in range(2):
                             d = 2 * dh + dj
                             for j in range(4):
                                 nc.tensor.transpose(
                                     ps[:, dj, j * 128:(j + 1) * 128],
                                     xt[:, j, d * 128:(d + 1) * 128],
                                     ident,
                                 )
                         for dj in range(2):
                             nc.vector.tensor_copy(
                                 xT[2 * dh + dj][:, g * 512:(g + 1) * 512],
                                 ps[:, dj, :],
                             )

             def emit_v(t):
                 pv = pj_ps.tile([128, CS], f32, tag="pp", name="ppv")
                 for k in range(KT):
                     nc.tensor.matmul(
                         pv,
                         xT[k][:, t * 128:(t + 1) * 128],
                         wv_sb[:, k, :],
                         start=(k == 0), stop=(k == KT - 1),
                     )
                 v3 = vA[t].rearrange("p (h c) -> p h c", c=VW)
                 nc.vector.tensor_copy(
                     v3[:, :, 0:DH], pv.rearrange("p (h d) -> p h d", d=DH)
                 )
                 nc.vector.tensor_copy(v3[:, :, DH:VW], ones4)

             def emit_q(p, ib):
                 pq = pj_ps.tile([128, 512], f32, tag="pp", name="pp")
                 for k in range(KT):
                     nc.tensor.matmul(
                         pq,
                         wq_sb[:, k, p * 128:(p + 1) * 128],
                         xT[k][:, ib * 512:(ib + 1) * 512],
                         start=(k == 0), stop=(k == KT - 1),
                     )
                 nc.vector.tensor_copy(qT[p][:, ib * 512:(ib + 1) * 512], pq)

             def emit_k(p, ib):
                 pk = pj_ps.tile([128, 512], f32, tag="pp", name="pp")
                 for k in range(KT):
                     nc.tensor.matmul(
                         pk,
                         wk_sb[:, k, p * 128:(p + 1) * 128],
                         xT[k][:, ib * 512:(ib + 1) * 512],
                         start=(k == 0), stop=(k == KT - 1),
                     )
                 nc.vector.tensor_copy(kTt[p][:, ib * 512:(ib + 1) * 512], pk)

             for ib in range(NIB):
                 emit_q(0, ib)
                 emit_k(0, ib)
             for t in range(4):
                 emit_v(t)

             # side work to interleave into pair-0 attention: 2 v-blocks per
             # j-group first (keeps 2 key-tiles ahead of the attn@v consumers),
             # then the pair-1 q/k projection blocks
             side_ops = [(lambda t=t: emit_v(t)) for t in range(4, NT)]
             for ib in range(NIB):
                 side_ops.append(lambda ib=ib: emit_q(1, ib))
                 side_ops.append(lambda ib=ib: emit_k(1, ib))

             # ---- attention (pair 0 with interleaved side work, then pair 1) ----
             with ExitStack() as aes:
                 aps = ExitStack()
                 sc_ps = aps.enter_context(tc.tile_pool(name="scps", bufs=2, space=PSUM))
                 ctx_ps = aps.enter_context(tc.tile_pool(name="ctxps", bufs=2, space=PSUM))
                 exp_pool = aes.enter_context(tc.tile_pool(name="expp", bufs=3))
                 stag_pool = aes.enter_context(tc.tile_pool(name="stagp", bufs=1))
                 evp = aes.enter_context(tc.tile_pool(name="evp", bufs=1))
                 bc_pool = aes.enter_context(tc.tile_pool(name="bcp", bufs=2))

                 for p in range(2):
                     lh0, lh1 = 2 * p, 2 * p + 1
                     # odd-head ctx rows 0:63 (cols ib*512) + odd-head dens row 64
                     stag = stag_pool.tile([65, 2048], f32r, tag="stag", name="stag")
                     # even-head denominators land on psum partition 64; collected
                     # at cols ib*512 of row 64 here
                     evden = evp.tile([65, 2048], f32r, tag="evden", name="evden")
                     for ib in range(NIB):
                         ibs = slice(ib * 512, (ib + 1) * 512)
                         if p == 1 or not side_ops:
                             # the projection psum pool is idle once the side
                             # work drains; borrowing a slot double-buffers
                             # the ctx accumulators across query blocks
                             c0 = pj_ps.tile([128, 512], f32, tag="pp", name="ctxb")
                         else:
                             c0 = ctx_ps.tile([128, 512], f32, tag="ctx", name="ctx")
                         c1 = ctx_ps.tile([128, 512], f32, tag="ctx", name="ctx")
                         for jg in range(NT // 2):
                             sA = sc_ps.tile([128, 1024], f32, tag="sc", name="sc")
                             sB = sc_ps.tile([128, 1024], f32, tag="sc", name="sc")
                             for jj in range(2):
                                 jt = 2 * jg + jj
                                 js = slice(jt * 128, (jt + 1) * 128)
                                 os_ = slice(jj * 512, (jj + 1) * 512)
                                 nc.tensor.matmul(
                                     sA[:, os_],
                                     kTt[p][0:64, js],
                                     qT[p][0:64, ibs],
                                     start=True, stop=True,
                                 )
                                 nc.tensor.matmul(
                                     sB[:, os_],
                                     kTt[p][64:128, js],
                                     qT[p][64:128, ibs],
                                     start=True, stop=True,
                                 )
                             if p == 0:
                                 # feed side work: 2 blocks while v remains (to
                                 # stay ahead of the attn@v reads), else 1
                                 n_feed = 2 if side_ops and len(side_ops) > 8 else 1
                                 for _ in range(n_feed):
                                     if side_ops:
                                         side_ops.pop(0)()
                             e0 = exp_pool.tile([128, 1024], f32r, tag="exp", name="exp")
                             e1 = exp_pool.tile([128, 1024], f32r, tag="exp", name="exp")
                             nc.scalar.activation(e0, sA, Exp, scale=SCALE)
                             nc.scalar.activation(e1, sB, Exp, scale=SCALE)
                             for jj in range(2):
                                 jt = 2 * jg + jj
                                 os_ = slice(jj * 512, (jj + 1) * 512)
                                 nc.tensor.matmul(
                                     c0[0:VW, :],
                                     vA[jt][:, lh0 * VW:(lh0 + 1) * VW],
                                     e0[:, os_],
                                     start=(jt == 0), stop=(jt == NT - 1),
                                 )
                                 nc.tensor.matmul(
                                     c1[0:VW, :],
                                     vA[jt][:, lh1 * VW:(lh1 + 1) * VW],
                                     e1[:, os_],
                                     start=(jt == 0), stop=(jt == NT - 1),
                                 )
                         # flush: even head straight to ctxT (same partitions);
                         # odd head + dens to staging (cross-partition moves and
                         # PSUM reads need an SBUF bounce)
                         nc.vector.tensor_copy(ctxT[p][0:64, ibs], c0[0:64, :])
                         nc.vector.tensor_copy(
                             evden[64:65, ib * 512:(ib + 1) * 512], c0[64:65, :]
                         )
                         nc.vector.tensor_copy(stag[0:65, ibs], c1[0:65, :])
                     if p == 0:
                         while side_ops:
                             side_ops.pop(0)()
                     # batched flush DMAs for this head pair, then this pair's
                     # softmax-denominator reciprocals + ctxT normalization --
                     # pair 0's chain overlaps pair 1's attention
                     nc.sync.dma_start(out=ctxT[p][64:128, :], in_=stag[0:64, :])
                     nc.sync.dma_start(
                         out=den_r[2 * p + 1:2 * p + 2, :], in_=stag[64:65, :]
                     )
                     nc.sync.dma_start(
                         out=den_r[2 * p:2 * p + 1, :], in_=evden[64:65, :]
                     )
                     rows = slice(8 * p, 8 * p + 8)
                     nc.sync.dma_start(out=den_all[p][:, :], in_=den_d[rows, :])
                     with nc.allow_low_precision(reason="f32r softmax denom"):
                         nc.vector.reciprocal(den_rec[p][:, :], den_all[p][:, :])
                     nc.sync.dma_start(out=den_d2[rows, :], in_=den_rec[p][:, :])
                     for ib in range(NIB):
                         ibs = slice(ib * 512, (ib + 1) * 512)
                         r0 = (2 * p) * NIB + ib
                         r1 = (2 * p + 1) * NIB + ib
                         bc = bc_pool.tile([128, 512], f32r, tag="bc", name="bc")
                         nc.sync.dma_start(
                             out=bc[0:64, :],
                             in_=den_d2[r0:r0 + 1, :].to_broadcast((64, 512)),
                         )
                         nc.sync.dma_start(
                             out=bc[64:128, :],
                             in_=den_d2[r1:r1 + 1, :].to_broadcast((64, 512)),
                         )
                         nc.vector.tensor_mul(ctxT[p][:, ibs], ctxT[p][:, ibs], bc)
                 aps.close()

             # ---- output projection ----
             with ExitStack() as oes:
                 if True:
                     o_ps = oes.enter_context(tc.tile_pool(name="ops", bufs=4, space=PSUM))
                     o_sb = oes.enter_context(tc.tile_pool(name="osb", bufs=2))
                     for q in range(NT // 2):
                         ot = o_sb.tile([128, 2, D], f32, tag="ot", name="ot")
                         for i4 in range(2):
                             it = q * 2 + i4
                             its = slice(it * 128, (it + 1) * 128)
                             for eh in range(2):
                                 ehs = slice(eh * 512, (eh + 1) * 512)
                                 po = o_ps.tile([128, 512], f32, tag="po", name="po")
                                 for cp in range(2):
                                     nc.tensor.matmul(
                                         po,
                                         ctxT[cp][:, its],
                                         wo_sb[:, cp, ehs],
                                         start=(cp == 0), stop=False,
                                     )
                                 nc.tensor.matmul(
                                     po,
                                     ones_col[0:1, :],
                                     bo_sb[0:1, ehs],
                                     start=False, stop=True,
                                 )
                                 if (it * 2 + eh) % 2 == 0:
                                     nc.scalar.copy(ot[:, i4, ehs], po)
                                 else:
                                     nc.vector.tensor_copy(ot[:, i4, ehs], po)
                         nc.sync.dma_start(out=out_g2[q], in_=ot)

             xps.close()

    nc.compile()
    return nc


def get_nc():
    if "nc" not in _CACHE:
        _CACHE["nc"] = _build_nc()
    return _CACHE["nc"]


def make_in_maps(x, Wq, Wk, Wv, Wo, bo):
    x = np.ascontiguousarray(np.asarray(x, dtype=np.float32))
    Wq = np.asarray(Wq, dtype=np.float32)
    Wk = np.asarray(Wk, dtype=np.float32)
    Wv = np.asarray(Wv, dtype=np.float32)
    Wo = np.asarray(Wo, dtype=np.float32)
    bo = np.asarray(bo, dtype=np.float32)
    zeros_bo = np.zeros((1, D), np.float32)
    in_maps = []
    for c in range(NCORES):
        b, hg = c // 4, c % 4
        sl = slice(hg * CS, (hg + 1) * CS)
        in_maps.append({
            "x": x[b],
            "wq": np.ascontiguousarray(Wq[:, sl]),
            "wk": np.ascontiguousarray(Wk[:, sl]),
            "wv": np.ascontiguousarray(Wv[:, sl]),
            "wo": np.ascontiguousarray(Wo[sl, :]),
            "bo": bo.reshape(1, D) if hg == 0 else zeros_bo,
        })
    return in_maps


def combine_outputs(results):
    outs = [np.asarray(r["out"], dtype=np.float64) for r in results]
    full = np.stack([
        outs[0] + outs[1] + outs[2] + outs[3],
        outs[4] + outs[5] + outs[6] + outs[7],
    ])
    return full.astype(np.float32)


def kernel(x, Wq, Wk, Wv, Wo, bo):
    from concourse.bass_utils import run_bass_kernel_spmd

    nc = get_nc()
    in_maps = make_in_maps(x, Wq, Wk, Wv, Wo, bo)
    res = run_bass_kernel_spmd(nc, in_maps, list(range(NCORES)))
    return combine_outputs(res.results)



# revision 24
# speedup vs baseline: 281.1647x; 281.1647x over previous
"""Multi-head self-attention (B=2, N=2048, D=1024, H=16, dh=64) on 8 trn2 cores.

Sharding: core c -> batch b = c // 4, head-group hg = c % 4 (4 heads per core).
Each core computes partial = Attn_{heads hg}(x_b) @ Wo[rows hg]; the host sums
the 4 partials per batch and adds bo (the unshard step).

Per-core pipeline (matmuls in f32r = full-rate fp32 on the PE):
  1. PE-transpose x_b -> xT (D on partitions), interleaved with per-group
     q/k projections for pair 0 and v tiles 0..3
  2. attention per head pair (2 heads packed in disjoint PE row groups so the
     score matmuls can run concurrently on silicon), per query block of 512,
     per key tile of 128:
       scoresT (2 MMs) -> exp on ScalarE (scale folded in, one [128,1024]
       instr covering both heads) -> attn@v accumulation in PSUM (v carries a
       ones column per head emitting softmax denominators on psum row 64)
     Pair-0 window is filled with v tiles 4..15 + pair-1 q/k projections;
     pair-1 window is filled with the output projection of completed query
     blocks.  Denominator reciprocals on DVE, broadcast to 128 partitions on
     GPSIMD, ctxT normalized in place on GPSIMD.
  3. out tiles = ctxT.T @ Wo via 2-step PSUM accumulation, DVE copy, DMA out.
"""

import numpy as np

B, N, D = 2, 2048, 1024
H, DH = 16, 64
HPC = 4                # heads per core
CS = HPC * DH          # 256 = per-core slice of the inner dim
NCORES = 8
SCALE = DH ** -0.5

NT = N // 128          # 16 token tiles
KT = D // 128          # 8 contraction tiles
NIB = N // 512         # 4 query blocks
VW = DH + 1            # 65: v plus ones column

_CACHE = {}


def _build_nc(reps=1):
    import concourse.bass as bass
    import concourse.bacc as bacc
    import concourse.mybir as mybir
    import concourse.tile as tile
    from contextlib import ExitStack

    f32 = mybir.dt.float32
    f32r = mybir.dt.float32r
    bf16 = mybir.dt.bfloat16
    PSUM = bass.MemorySpace.PSUM
    Exp = mybir.ActivationFunctionType.Exp

    nc = bacc.Bacc()

    x_d = nc.dram_tensor("x", [N, D], f32r, kind="ExternalInput")
    wq_d = nc.dram_tensor("wq", [D, CS], f32r, kind="ExternalInput")
    wk_d = nc.dram_tensor("wk", [D, CS], f32r, kind="ExternalInput")
    wv_d = nc.dram_tensor("wv", [D, CS], f32r, kind="ExternalInput")
    wo_d = nc.dram_tensor("wo", [CS, D], f32r, kind="ExternalInput")
    out_d = nc.dram_tensor("out", [N, D], f32, kind="ExternalOutput")
    den_d = nc.dram_tensor("den_scratch", [16, 512], f32r)

    ident_d = nc.inline_tensor(np.eye(128, dtype=np.float32), name="ident")

    # grouped views for batched DMAs
    x_g = x_d.rearrange("(g j p) d -> g p j d", p=128, j=2)        # [8][128,2,1024]
    wq_g = wq_d.rearrange("(k p) c -> p k c", p=128)               # [128,8,256]
    wk_g = wk_d.rearrange("(k p) c -> p k c", p=128)
    wv_g = wv_d.rearrange("(k p) c -> p k c", p=128)
    wo_g = wo_d.rearrange("(k p) c -> p k c", p=128)               # [128,2,1024]
    out_g2 = out_d.rearrange("(q i p) e -> q p i e", p=128, i=2)   # [8][128,2,1024]

    with tile.TileContext(nc) as tc:
      for _rep in range(reps):
       with ExitStack() as es:
             singles = es.enter_context(tc.tile_pool(name="singles", bufs=1))

             ones4 = singles.tile([128, HPC, 1], bf16, tag="ones4")
             nc.vector.memset(ones4, 1.0)

             wq_sb = singles.tile([128, KT, CS], f32r, tag="wq")
             wk_sb = singles.tile([128, KT, CS], f32r, tag="wk")
             wv_sb = singles.tile([128, KT, CS], f32r, tag="wv")
             wo_sb = singles.tile([128, 2, D], f32r, tag="wo")

             qT = [singles.tile([128, N], f32r, tag=f"qT{p}", name=f"qT{p}") for p in range(2)]
             kTt = [singles.tile([128, N], f32r, tag=f"kT{p}", name=f"kT{p}") for p in range(2)]
             vA = [singles.tile([128, HPC * VW], bf16, tag=f"v{t}", name=f"v{t}") for t in range(NT)]
             ctxT = [singles.tile([128, N], f32r, tag=f"ctxT{p}", name=f"ctxT{p}") for p in range(2)]
             o_sb = es.enter_context(tc.tile_pool(name="osb", bufs=2))
             rec_pool = es.enter_context(tc.tile_pool(name="recp", bufs=2))

             # ---- phase 1: transpose x; interleave pair-0 q/k and v[0:4] ----
             xps = ExitStack()
             xT_pool = xps.enter_context(tc.tile_pool(name="xTp", bufs=KT))
             pj_ps = xps.enter_context(tc.tile_pool(name="pjps", bufs=2, space=PSUM))
             xT = [xT_pool.tile([128, N], f32r, tag="xT", name="xT") for _ in range(KT)]

             def emit_v(t):
                 pv = pj_ps.tile([128, CS], f32, tag="pp", name="ppv")
                 for k in range(KT):
                     nc.tensor.matmul(
                         pv,
                         xT[k][:, t * 128:(t + 1) * 128],
                         wv_sb[:, k, :],
                         start=(k == 0), stop=(k == KT - 1),
                     )
                 v3 = vA[t].rearrange("p (h c) -> p h c", c=VW)
                 nc.vector.tensor_copy(
                     v3[:, :, 0:DH], pv.rearrange("p (h d) -> p h d", d=DH)
                 )
                 nc.vector.tensor_copy(v3[:, :, DH:VW], ones4)

             pending_pq = {}

             def emit_qk(dst, w_sb, p, ib, half):
                 # half 0/1: contraction tiles 0..3 / 4..7 (split so side ops
                 # stay ~2k cycles each; both halves accumulate into one tile)
                 key = (id(dst), p, ib)
                 if half == 0:
                     pq = pj_ps.tile([128, 512], f32, tag="pp", name="pp")
                     pending_pq[key] = pq
                 else:
                     pq = pending_pq.pop(key)
                 for kk in range(4):
                     k = half * 4 + kk
                     nc.tensor.matmul(
                         pq,
                         w_sb[:, k, p * 128:(p + 1) * 128],
                         xT[k][:, ib * 512:(ib + 1) * 512],
                         start=(k == 0), stop=(k == KT - 1),
                     )
                 if half == 1:
                     nc.vector.tensor_copy(dst[p][:, ib * 512:(ib + 1) * 512], pq)

             with ExitStack() as pes:
                 idp = pes.enter_context(tc.tile_pool(name="idp", bufs=1))
                 x_pool = pes.enter_context(tc.tile_pool(name="xp", bufs=2))
                 tp_ps = pes.enter_context(tc.tile_pool(name="tpps", bufs=2, space=PSUM))

                 ident = idp.tile([128, 128], f32r, tag="ident")
                 nc.sync.dma_start(out=ident, in_=ident_d[:, :].bitcast(f32r))

                 wdma = {
                     1: lambda: nc.sync.dma_start(out=wq_sb, in_=wq_g),
                     2: lambda: nc.sync.dma_start(out=wk_sb, in_=wk_g),
                     3: lambda: nc.sync.dma_start(out=wv_sb, in_=wv_g),
                     6: lambda: nc.sync.dma_start(out=wo_sb, in_=wo_g),
                 }
                 # pair-0 projection/v emissions paced so each slot's weight
                 # DMA (fired at g=1..4) has landed well before first use
                 prologue_emits = {
                     1: [lambda: emit_qk(qT, wq_sb, 0, 0, 0),
                         lambda: emit_qk(qT, wq_sb, 0, 0, 1)],
                     3: [lambda: emit_qk(kTt, wk_sb, 0, 0, 0),
                         lambda: emit_qk(kTt, wk_sb, 0, 0, 1),
                         lambda: emit_qk(qT, wq_sb, 0, 1, 0),
                         lambda: emit_qk(qT, wq_sb, 0, 1, 1)],
                     5: [lambda: emit_qk(kTt, wk_sb, 0, 1, 0),
                         lambda: emit_qk(kTt, wk_sb, 0, 1, 1),
                         lambda: emit_qk(qT, wq_sb, 0, 2, 0),
                         lambda: emit_qk(qT, wq_sb, 0, 2, 1),
                         lambda: emit_v(0), lambda: emit_v(1)],
                     7: [lambda: emit_qk(kTt, wk_sb, 0, 2, 0),
                         lambda: emit_qk(kTt, wk_sb, 0, 2, 1),
                         lambda: emit_qk(qT, wq_sb, 0, 3, 0),
                         lambda: emit_qk(qT, wq_sb, 0, 3, 1),
                         lambda: emit_v(2), lambda: emit_v(3)],
                 }
                 for g in range(NT // 2):  # 8 groups of 2 token tiles
                     xt = x_pool.tile([128, 2, D], f32r, tag="x", name="xt")
                     nc.sync.dma_start(out=xt, in_=x_g[g])
                     if g in wdma:
                         wdma[g]()
                     for dh in range(KT // 2):
                         ps = tp_ps.tile([128, 2, 256], f32r, tag="tp", name="tp")
                         for dj in range(2):
                             d = 2 * dh + dj
                             for j in range(2):
                                 nc.tensor.transpose(
                                     ps[:, dj, j * 128:(j + 1) * 128],
                                     xt[:, j, d * 128:(d + 1) * 128],
                                     ident,
                                 )
                         for dj in range(2):
                             nc.vector.tensor_copy(
                                 xT[2 * dh + dj][:, g * 256:(g + 1) * 256],
                                 ps[:, dj, :],
                             )
                     for op in prologue_emits.get(g, []):
                         op()

             # side work queues: window 0 gets remaining v tiles + pair-1
             # projections; window 1 gets the output projection (pushed
             # per completed query block)
             side_ops = [
                 lambda: emit_qk(kTt, wk_sb, 0, 3, 0),
                 lambda: emit_qk(kTt, wk_sb, 0, 3, 1),
             ]
             side_ops += [(lambda t=t: emit_v(t)) for t in range(4, NT)]
             for ib in range(NIB):
                 side_ops.append(lambda ib=ib: emit_qk(qT, wq_sb, 1, ib, 0))
                 side_ops.append(lambda ib=ib: emit_qk(qT, wq_sb, 1, ib, 1))
                 side_ops.append(lambda ib=ib: emit_qk(kTt, wk_sb, 1, ib, 0))
                 side_ops.append(lambda ib=ib: emit_qk(kTt, wk_sb, 1, ib, 1))

             def emit_out(it, eh):
                 # out tile [128 tok, 512 D] for token tile `it`, D half `eh`
                 ehs = slice(eh * 512, (eh + 1) * 512)
                 po = pj_ps.tile([128, 512], f32, tag="pp", name="po")
                 for cp in range(2):
                     nc.tensor.matmul(
                         po,
                         ctxT[cp][:, it * 128:(it + 1) * 128],
                         wo_sb[:, cp, ehs],
                         start=(cp == 0), stop=(cp == 1),
                     )
                 ot = ot_tiles[it // 2]
                 nc.vector.tensor_copy(ot[:, it % 2, ehs], po)

             ot_tiles = {}

             def flush_out(q):
                 nc.sync.dma_start(out=out_g2[q], in_=ot_tiles[q])

             # ---- attention ----
             with ExitStack() as aes:
                 sc_ps = aes.enter_context(tc.tile_pool(name="scps", bufs=2, space=PSUM))
                 ctx_ps = aes.enter_context(tc.tile_pool(name="ctxps", bufs=2, space=PSUM))
                 exp_pool = aes.enter_context(tc.tile_pool(name="expp", bufs=3))
                 bc_pool = aes.enter_context(tc.tile_pool(name="bcp", bufs=2))

                 out_q = []  # deferred out-projection ops (window 1)

                 for p in range(2):
                     lh0, lh1 = 2 * p, 2 * p + 1
                     for ib in range(NIB):
                         ibs = slice(ib * 512, (ib + 1) * 512)
                         c0 = ctx_ps.tile([65, 512], f32, tag="ctx", name="ctx0")
                         c1 = ctx_ps.tile([65, 512], f32, tag="ctx", name="ctx1")

                         def av(jt, e):
                             nc.tensor.matmul(
                                 c0,
                                 vA[jt][:, lh0 * VW:(lh0 + 1) * VW],
                                 e[:, 0, :],
                                 start=(jt == 0), stop=(jt == NT - 1),
                             )
                             nc.tensor.matmul(
                                 c1,
                                 vA[jt][:, lh1 * VW:(lh1 + 1) * VW],
                                 e[:, 1, :],
                                 start=(jt == 0), stop=(jt == NT - 1),
                             )

                         # software-pipelined: av for tile jt-1 runs on PE
                         # while ScalarE computes exp for tile jt
                         prev = None
                         for jt in range(NT):
                             js = slice(jt * 128, (jt + 1) * 128)
                             sc = sc_ps.tile([128, 2, 512], f32, tag="sc", name="sc")
                             nc.tensor.matmul(
                                 sc[:, 0, :],
                                 kTt[p][0:64, js],
                                 qT[p][0:64, ibs],
                                 start=True, stop=True,
                             )
                             nc.tensor.matmul(
                                 sc[:, 1, :],
                                 kTt[p][64:128, js],
                                 qT[p][64:128, ibs],
                                 start=True, stop=True,
                             )
                             # pace fill work into the ACT-bound pipeline;
                             # early side ops (k03, v tiles) must stay ahead
                             # of their consumers, so feed those 1/ktile
                             if p == 0:
                                 if side_ops and (len(side_ops) > 16 or jt % 2 == 0):
                                     side_ops.pop(0)()
                             else:
                                 if out_q:
                                     out_q.pop(0)()
                                 if ib == NIB - 1 and out_q:
                                     out_q.pop(0)()
                             e = exp_pool.tile([128, 2, 512], bf16, tag="exp", name="exp")
                             nc.scalar.activation(
                                 e.rearrange("p a b -> p (a b)"),
                                 sc.rearrange("p a b -> p (a b)"),
                                 Exp, scale=SCALE,
                             )
                             if prev is not None:
                                 av(*prev)
                             prev = (jt, e)
                         av(*prev)
                         # two-step flush: free c0/c1 fast (unnormalized), then
                         # normalize ctxT in place on GPSIMD once recips land
                         nc.vector.tensor_copy(ctxT[p][0:64, ibs], c0[0:64, :])
                         nc.vector.tensor_copy(ctxT[p][64:128, ibs], c1[0:64, :])
                         rec2 = rec_pool.tile([33, 512], f32r, tag="rec", name="rec")
                         r0 = rec2[0:1, :]
                         r1 = rec2[32:33, :]
                         with nc.allow_low_precision(reason="f32r softmax denom"):
                             nc.vector.reciprocal(r0, c0[64:65, :])
                             nc.vector.reciprocal(r1, c1[64:65, :])
                         ri = 4 * p + ib
                         nc.sync.dma_start(out=den_d[2*ri:2*ri+1, :], in_=r0)
                         nc.sync.dma_start(out=den_d[2*ri+1:2*ri+2, :], in_=r1)
                         bc = bc_pool.tile([128, 512], f32r, tag="bc", name="bc")
                         nc.sync.dma_start(
                             out=bc[0:64, :],
                             in_=den_d[2*ri:2*ri+1, :].to_broadcast((64, 512)))
                         nc.sync.dma_start(
                             out=bc[64:128, :],
                             in_=den_d[2*ri+1:2*ri+2, :].to_broadcast((64, 512)))
                         nc.vector.tensor_mul(ctxT[p][:, ibs], ctxT[p][:, ibs], bc)
                         # queue this block's output projection for window 1
                         # (needs both pairs' ctxT for these tokens)
                         if p == 0:
                             continue
                         for itl in range(4 * ib, 4 * ib + 4):
                             if itl % 2 == 0:
                                 def mk(itl=itl):
                                     ot_tiles[itl // 2] = o_sb.tile(
                                         [128, 2, D], f32, tag="ot", name="ot"
                                     )
                                 out_q.append(mk)
                             for eh in range(2):
                                 out_q.append(lambda itl=itl, eh=eh: emit_out(itl, eh))
                             if itl % 2 == 1:
                                 out_q.append(lambda itl=itl: flush_out(itl // 2))
                     if p == 0:
                         while side_ops:
                             side_ops.pop(0)()

                 # tail: drain remaining output projection work
                 while out_q:
                     out_q.pop(0)()

             xps.close()

    nc.compile()
    return nc


def get_nc():
    if "nc" not in _CACHE:
        _CACHE["nc"] = _build_nc()
    return _CACHE["nc"]


def make_in_maps(x, Wq, Wk, Wv, Wo, bo):
    x = np.ascontiguousarray(np.asarray(x, dtype=np.float32))
    Wq = np.asarray(Wq, dtype=np.float32)
    Wk = np.asarray(Wk, dtype=np.float32)
    Wv = np.asarray(Wv, dtype=np.float32)
    Wo = np.asarray(Wo, dtype=np.float32)
    in_maps = []
    for c in range(NCORES):
        b, hg = c // 4, c % 4
        sl = slice(hg * CS, (hg + 1) * CS)
        in_maps.append({
            "x": x[b],
            "wq": np.ascontiguousarray(Wq[:, sl]),
            "wk": np.ascontiguousarray(Wk[:, sl]),
            "wv": np.ascontiguousarray(Wv[:, sl]),
            "wo": np.ascontiguousarray(Wo[sl, :]),
        })
    return in_maps


def combine_outputs(results, bo):
    outs = [np.asarray(r["out"], dtype=np.float64) for r in results]
    full = np.stack([
        outs[0] + outs[1] + outs[2] + outs[3],
        outs[4] + outs[5] + outs[6] + outs[7],
    ]) + np.asarray(bo, dtype=np.float64)
    return full.astype(np.float32)


def kernel(x, Wq, Wk, Wv, Wo, bo):
    from concourse.bass_utils import run_bass_kernel_spmd

    nc = get_nc()
    in_maps = make_in_maps(x, Wq, Wk, Wv, Wo, bo)
    res = run_bass_kernel_spmd(nc, in_maps, list(range(NCORES)))
    return combine_outputs(res.results, bo)
